# revision 2
# baseline (speedup 1.0000x reference)
"""Bahdanau-attention scores kernel for one TRN2 chip (8 NeuronCores).

Reference computation (B=32, S=2048, H=1024):
    energy = tanh(hidden @ W1^T + enc @ W2^T + b)   # (B, S, H)
    scores = energy . v                             # (B, S)
    out    = softmax(scores, axis=S)[:, None, :]    # (B, 1, S)

Distribution: data-parallel over B - each of the 8 cores handles 4 batch
rows; small tensors (attn_W, attn_b, v, hidden) replicated. No collectives.

Numerics: the enc @ W2^T contraction runs entirely in fp8 (e4m3) with
DoubleRow matmuls (2x PE throughput vs bf16). fp8 weights are pre-scaled
by WS=16 (avoids the subnormal region) and the scale is undone by the tanh
activation's input scale; the hidden term is computed exactly on the host.

Per-core layout (pre-packed on host so DMAs are contiguous):
    e8    (BL, NG, P, 4, 2, GW) fp8   enc in DoubleRow layout, h = blk*256+j*128+p
    e8a   (8, P, 2, 512)       fp8    first-group enc pieces
    w8    (P, K8, 4, 2, P)     fp8    W2^T * WS, DR layout per (kt, blk)
    hb    (P, 8, BL)           f32    hidden @ W1^T + attn_b (host, exact)
    vvs   (P, 8) f32                  v tiled (p, kt)
    oneh  (P, 2, 2) bf16              one-hot columns e0 / e1
    vht   (P, 2, 2, 2) bf16           v-column one-hot [kt-6][sc] -> [P,2]
    on2f  (2, 2) f32                  ones (partition-pair total matmul)

On-core dataflow per (b, g) group (GW=1024 score columns):
    for kt: psum[k=128, GW] = sum_h w[h,k] enc[h,s]   (clean-mode PE streams)
            th = tanh(psum * (1/WS) + hb[k,b])        (ScalarE, bf16 out)
            acc = th * v[k] + acc                     (DVE fused, f32;
                                                       last kt emits bf16)
    pv[2, 512] per group: one-hot stationary e_sc routes the partition-sum
    of each 512-col half to psum row sc (deferred one group so the PE never
    waits on the DVE chain); ONE exp [2,512] per group with accum_out.
    Last group: accb stops at kt5; kt6/kt7 go tanh-half -> v-one-hot matmul
    straight into pv, so the tail never waits on the DVE chain.
    finish_b: DVE free-reduce of sm2 -> tiny fp32 PE matmul for the
    partition-pair total -> reciprocal -> one [2,1024] scale -> 2 out DMAs.
"""

import numpy as np

B, S, H = 32, 2048, 1024
NCORES = 8
BL = B // NCORES          # batch rows per core
P = 128                   # SBUF partitions
KT = 8                    # k-tiles
K8 = KT
GW = 1024                 # score columns per group (2 PSUM banks)
NG = S // GW              # groups per batch row
WS = 16.0                 # fp8 weight pre-scale

_CACHE = {}


def _build_nc():
    import concourse.bacc as bacc
    import concourse.mybir as mybir
    import concourse.tile as tile

    dt = mybir.dt
    AFT = mybir.ActivationFunctionType
    DR = mybir.MatmulPerfMode.DoubleRow

    nc = bacc.Bacc("TRN2", target_bir_lowering=False, debug=False)

    e8_d = nc.declare_dram_parameter("e8", [BL, NG, P, 4, 2, GW], dt.float8e4, isOutput=False)
    e8a_d = nc.declare_dram_parameter("e8a", [2, 2, P, 2, 2, 512], dt.float8e4, isOutput=False)
    w8_d = nc.declare_dram_parameter("w8", [P, K8, 4, 2, P], dt.float8e4, isOutput=False)
    hb_d = nc.declare_dram_parameter("hb", [P, KT, BL], dt.float32, isOutput=False)
    vvs_d = nc.declare_dram_parameter("vvs", [P, KT], dt.float32, isOutput=False)
    oneh_d = nc.declare_dram_parameter("oneh", [P, 2, 2], dt.bfloat16, isOutput=False)
    vht_d = nc.declare_dram_parameter("vht", [P, 2, 2, 2], dt.bfloat16, isOutput=False)
    on2f_d = nc.declare_dram_parameter("on2f", [2, 2], dt.float32, isOutput=False)
    out_d = nc.declare_dram_parameter("out", [BL, NG, 2, 512], dt.float32, isOutput=True)

    with tile.TileContext(nc) as tc:
        with (
            tc.tile_pool(name="const", bufs=1) as constp,
            tc.tile_pool(name="enc8", bufs=4) as encp8,
            tc.tile_pool(name="enc0", bufs=1) as encp0,
            tc.tile_pool(name="tanh", bufs=4) as tanhp,
            tc.tile_pool(name="accp", bufs=3) as accp,
            tc.tile_pool(name="soft", bufs=2) as softp,
            tc.tile_pool(name="pe", bufs=3, space="PSUM") as pep,
            tc.tile_pool(name="pv", bufs=2, space="PSUM") as pvp,
        ):
            # scalar ring carries only what gates the first tanh (so its
            # queue frees early and the act-table load + tanh chain start
            # on time); everything else rides sync/gpsimd at its deadline
            w8 = constp.tile([P, K8, 4, 2, P], dt.float8e4)
            nc.scalar.dma_start(w8[:, 0], w8_d[:, 0])
            hb = constp.tile([P, KT, BL], dt.float32)
            nc.scalar.dma_start(hb[:], hb_d.ap())
            vvs = constp.tile([P, KT], dt.float32)
            nc.scalar.dma_start(vvs[:], vvs_d.ap())

            # group 0 runs as two 512-col half-passes so the startup demand
            # curve matches chip-level delivery: the first DR matmul needs
            # only 256 KB resident, and every later deadline has >=1 us of
            # margin. Sync-ring order interleaves the early w8 kts at their
            # consumption times.
            e8h = [
                encp0.tile([P, 4, 2, 512], dt.float8e4, tag=f"e8h{i}",
                           name=f"e8h{i}")
                for i in range(2)
            ]
            nc.sync.dma_start(e8h[0][:, 0:2], e8a_d[0][0])
            nc.gpsimd.dma_start(e8h[0][:, 2:4], e8a_d[0][1])
            nc.sync.dma_start(w8[:, 1], w8_d[:, 1])
            nc.sync.dma_start(w8[:, 2], w8_d[:, 2])
            nc.sync.dma_start(e8h[1][:, 0:2], e8a_d[1][0])
            nc.sync.dma_start(e8h[1][:, 2:4], e8a_d[1][1])
            for k8 in range(3, K8):
                nc.gpsimd.dma_start(w8[:, k8], w8_d[:, k8])
            oneh = constp.tile([P, 2, 2], dt.bfloat16)
            nc.gpsimd.dma_start(oneh[:], oneh_d.ap())
            vht = constp.tile([P, 2, 2, 2], dt.bfloat16)
            nc.gpsimd.dma_start(vht[:], vht_d.ap())
            on2f = constp.tile([2, 2], dt.float32)
            nc.gpsimd.dma_start(on2f[:], on2f_d.ap())

            # PE warm-up (clock ramp) while the first DMAs land: cheap
            # [P,1]x[P,512] matmuls into a pe-pool tile.
            wut = constp.tile([P, GW], dt.bfloat16, tag="wut")
            nc.vector.memset(wut[:], 0.0)
            wps = pep.tile([P, GW], dt.float32, tag="ps", name="wps")
            for i in range(6):
                nc.tensor.matmul(
                    wps[0:1, 0:512], wut[:, 0:1],
                    wut[:, 0:512], start=True, stop=True,
                )

            ex_tiles = {}
            sm_tiles = {}
            pending = None

            def finish_group(p):
                pb, pg, pmov, ppv = p
                if ppv is None:
                    pv = pvp.tile([2, 512], dt.float32, tag="pv", name="pv")
                    for sc in range(2):
                        nc.tensor.matmul(
                            pv[:], oneh[:, sc], pmov[sc],
                            start=(sc == 0), stop=(sc == 1),
                        )
                else:
                    pv = ppv
                nc.scalar.activation(
                    ex_tiles[pb][:, pg], pv[:], AFT.Exp,
                    accum_out=sm_tiles[pb][:, pg:pg + 1],
                )

            def finish_b(pb):
                smr = softp.tile([2, 1], dt.float32, tag="smr")
                nc.vector.tensor_reduce(
                    smr[:], sm_tiles[pb][:], axis=mybir.AxisListType.X,
                    op=mybir.AluOpType.add,
                )
                t2 = pvp.tile([2, 1], dt.float32, tag="pv", name="t2")
                nc.tensor.matmul(t2[:], on2f[:], smr[:], start=True, stop=True)
                rc = softp.tile([2, 1], dt.float32, tag="rc")
                nc.vector.reciprocal(rc[:], t2[:])
                ot = softp.tile([2, NG, 512], dt.float32, tag="ot")
                nc.vector.tensor_scalar_mul(ot[:], ex_tiles[pb][:], rc[:])
                nc.gpsimd.dma_start(out_d[pb][0], ot[:, 0])
                nc.sync.dma_start(out_d[pb][1], ot[:, 1])

            for b in range(BL):
                ex_tiles[b] = softp.tile([2, NG, 512], dt.float32, tag="ex", name="ex")
                sm_tiles[b] = softp.tile([2, NG], dt.float32, tag="sm", name="sm")
                for g in range(NG):
                    tsc = 1.0 / WS
                    if b == 0 and g == 0:
                        # two 512-col half-passes (see startup comment)
                        movers = []
                        for hf in range(2):
                            acch = accp.tile([P, GW], dt.float32)
                            accbh = tanhp.tile([P, GW], dt.bfloat16,
                                               tag="accb", name=f"accbh{hf}")
                            for kt in range(KT):
                                ps = pep.tile([P, GW], dt.float32,
                                              tag="ps", name="ps")
                                for blk in range(4):
                                    nc.tensor.matmul(
                                        ps[:, 0:512], w8[:, kt, blk],
                                        e8h[hf][:, blk],
                                        start=(blk == 0), stop=(blk == 3),
                                        perf_mode=DR,
                                    )
                                th = tanhp.tile([P, GW], dt.bfloat16)
                                nc.scalar.activation(
                                    th[:, 0:512], ps[:, 0:512], AFT.Tanh,
                                    bias=hb[:, kt, 0:1], scale=tsc,
                                )
                                if kt == 0:
                                    nc.vector.tensor_scalar_mul(
                                        acch[:, 0:512], th[:, 0:512],
                                        vvs[:, 0:1])
                                else:
                                    dst = accbh if kt == KT - 1 else acch
                                    nc.vector.scalar_tensor_tensor(
                                        dst[:, 0:512], th[:, 0:512],
                                        vvs[:, kt:kt + 1], acch[:, 0:512],
                                        op0=mybir.AluOpType.mult,
                                        op1=mybir.AluOpType.add,
                                    )
                            movers.append(accbh[:, 0:512])
                        pending = (0, 0, movers, None)
                        continue
                    e8t = [
                        encp8.tile([P, 2, GW], dt.float8e4, tag=f"e8t{blk}",
                                   name=f"e8t{blk}")
                        for blk in range(4)
                    ]
                    for blk in range(4):
                        nc.sync.dma_start(e8t[blk][:], e8_d[b][g][:, blk])
                    last = (b == BL - 1 and g == NG - 1)
                    acc = accp.tile([P, GW], dt.float32)
                    accb = tanhp.tile([P, GW], dt.bfloat16, tag="accb", name="accb")
                    if last:
                        pvl = pvp.tile([2, 512], dt.float32, tag="pv", name="pvl")
                        ps6 = None
                    for kt in range(KT):
                        ps = pep.tile([P, GW], dt.float32, tag="ps", name="ps")
                        for blk in range(4):
                            for sc in range(2):
                                nc.tensor.matmul(
                                    ps[:, sc * 512:(sc + 1) * 512],
                                    w8[:, kt, blk],
                                    e8t[blk][:, :, sc * 512:(sc + 1) * 512],
                                    start=(blk == 0), stop=(blk == 3),
                                    perf_mode=DR,
                                )
                        if last and kt == KT - 2:
                            # defer: tanh+v-matmul emitted after kt7's DR
                            # stream so the PE never stalls on the tanh
                            ps6 = ps
                        elif last and kt == KT - 1:
                            # partition-sums of the kt0-5 chain (accb ready
                            # since kt5's DVE op, during kt6's DR stream)
                            for sc in range(2):
                                nc.tensor.matmul(
                                    pvl[:], oneh[:, sc],
                                    accb[:, sc * 512:(sc + 1) * 512],
                                    start=(sc == 0), stop=False,
                                    skip_group_check=True,
                                )
                            # kt6/kt7: one tanh each -> one-hot v-column
                            # matmuls straight into pvl rows
                            for kq, psq in ((KT - 2, ps6), (KT - 1, ps)):
                                thq = tanhp.tile([P, GW], dt.bfloat16,
                                                 tag="thh", name="thh")
                                nc.scalar.activation(
                                    thq[:], psq[:], AFT.Tanh,
                                    bias=hb[:, kq, b:b + 1], scale=tsc,
                                )
                                for sc in range(2):
                                    nc.tensor.matmul(
                                        pvl[:], vht[:, kq - (KT - 2), sc],
                                        thq[:, sc * 512:(sc + 1) * 512],
                                        start=False,
                                        stop=(kq == KT - 1 and sc == 1),
                                        skip_group_check=True,
                                    )
                        else:
                            th = tanhp.tile([P, GW], dt.bfloat16)
                            nc.scalar.activation(
                                th[:], ps[:], AFT.Tanh, bias=hb[:, kt, b:b + 1],
                                scale=tsc,
                            )
                            if kt == 0:
                                nc.vector.tensor_scalar_mul(
                                    acc[:], th[:], vvs[:, kt:kt + 1])
                            else:
                                last_chain = KT - 3 if last else KT - 1
                                dst = accb if kt == last_chain else acc
                                nc.vector.scalar_tensor_tensor(
                                    dst[:], th[:], vvs[:, kt:kt + 1], acc[:],
                                    op0=mybir.AluOpType.mult,
                                    op1=mybir.AluOpType.add,
                                )
                        if kt == 1 and pending is not None:
                            # deferred by a full kt so the pv matmuls never
                            # wait on the previous group's DVE chain
                            finish_group(pending)
                            if pending[1] == NG - 1:
                                finish_b(pending[0])
                            pending = None
                    if last:
                        finish_group((b, g, None, pvl))
                        finish_b(b)
                    else:
                        pending = (b, g, [accb[:, 0:512], accb[:, 512:GW]],
                                   None)

    nc.compile()
    return nc


def _get_nc():
    if "nc" not in _CACHE:
        _CACHE["nc"] = _build_nc()
    return _CACHE["nc"]


def _make_in_maps(hidden, encoder_outputs, attn_W, attn_b, v):
    import concourse.mybir as mybir

    bf16 = mybir.dt.np(mybir.dt.bfloat16)
    f8 = mybir.dt.np(mybir.dt.float8e4)
    f32 = np.float32

    order = np.argsort(-np.abs(v), kind="stable")
    W2p = attn_W[:, H:].T[:, order]          # (h, k) permuted columns
    vp = v[order]
    hid = hidden[0]                           # (B, H)
    hterm = (hid @ attn_W[:, :H].T + attn_b).astype(f32)[:, order]  # (B, k)

    w8 = np.ascontiguousarray(
        (W2p * WS).reshape(4, 2, P, K8, P).transpose(2, 3, 0, 1, 4)
    ).astype(f8)
    vvs = np.ascontiguousarray(vp.reshape(KT, P).T).astype(f32)
    vvb = vvs.astype(bf16)
    oneh = np.zeros((P, 2, 2), dtype=bf16)
    oneh[:, 0, 0] = 1.0
    oneh[:, 1, 1] = 1.0
    vht = np.zeros((P, 2, 2, 2), dtype=bf16)
    for i, kq in enumerate((KT - 2, KT - 1)):
        vht[:, i, 0, 0] = vvb[:, kq]
        vht[:, i, 1, 1] = vvb[:, kq]
    on2f = np.ones((2, 2), dtype=f32)

    shared = {"w8": w8, "vvs": vvs, "oneh": oneh, "vht": vht, "on2f": on2f}

    in_maps = []
    for c in range(NCORES):
        sl = slice(c * BL, (c + 1) * BL)
        encs = encoder_outputs[sl]            # (BL, S, H)
        e8 = np.ascontiguousarray(
            encs.reshape(BL, NG, GW, 4, 2, P).transpose(0, 1, 5, 3, 4, 2)
        ).astype(f8)
        # [sc, blkpair, P, blk%2, dr, col]
        e8a = np.ascontiguousarray(
            encs[0, :GW].reshape(2, 512, 2, 2, 2, P).transpose(0, 2, 5, 3, 4, 1)
        ).astype(f8)
        hbias = np.ascontiguousarray(
            hterm[sl].T.reshape(KT, P, BL).transpose(1, 0, 2)
        )
        m = {"e8": e8, "e8a": e8a, "hb": hbias, **shared}
        in_maps.append(m)
    return in_maps


def kernel(hidden, encoder_outputs, attn_W, attn_b, v):
    from concourse.bass_utils import run_bass_kernel_spmd

    nc = _get_nc()
    in_maps = _make_in_maps(
        np.asarray(hidden, dtype=np.float32),
        np.asarray(encoder_outputs, dtype=np.float32),
        np.asarray(attn_W, dtype=np.float32),
        np.asarray(attn_b, dtype=np.float32),
        np.asarray(v, dtype=np.float32),
    )
    # A freshly-opened device occasionally fails its first execution with
    # NRT_EXEC_UNIT_UNRECOVERABLE; a retry on the reset device succeeds.
    last_err = None
    for attempt in range(3):
        try:
            res = run_bass_kernel_spmd(nc, in_maps, core_ids=list(range(NCORES)))
            break
        except Exception as e:
            last_err = e
            import time
            time.sleep(2.0)
    else:
        raise last_err
    out = np.concatenate(
        [res.results[c]["out"].reshape(BL, S) for c in range(NCORES)], axis=0)
    return out[:, None, :].astype(np.float32)


# revision 3
# speedup vs baseline: 1.0256x; 1.0256x over previous
"""Bahdanau-attention scores kernel for one TRN2 chip (8 NeuronCores).

Reference computation (B=32, S=2048, H=1024):
    energy = tanh(hidden @ W1^T + enc @ W2^T + b)   # (B, S, H)
    scores = energy . v                             # (B, S)
    out    = softmax(scores, axis=S)[:, None, :]    # (B, 1, S)

Distribution: data-parallel over B - each of the 8 cores handles 4 batch
rows; small tensors (attn_W, attn_b, v, hidden) replicated. No collectives.

Numerics: the enc @ W2^T contraction runs entirely in fp8 (e4m3) with
DoubleRow matmuls (2x PE throughput vs bf16). fp8 weights are pre-scaled
by WS=16 (avoids the subnormal region) and the scale is undone by the tanh
activation's input scale; the hidden term is computed exactly on the host.

Per-core layout (pre-packed on host so DMAs are contiguous):
    e8    (BL, NG, P, 4, 2, GW) fp8   enc in DoubleRow layout, h = blk*256+j*128+p
    e8a   (2, 2, P, 2, 2, 512) fp8    group-0 enc, (sc, blkpair) pieces
    w8    (P, K8, 4, 2, P)     fp8    W2^T * WS, DR layout per (kt, blk)
    hb    (P, 8, BL)           f32    hidden @ W1^T + attn_b (host, exact)
    vvs   (P, 8) f32                  v tiled (p, kt)
    oneh  (P, 2, 2) bf16              one-hot columns e0 / e1
    vht   (P, 2, 2, 2) bf16           v-column one-hot [kt-6][sc] -> [P,2]
    on2f  (2, 2) f32                  ones (partition-pair total matmul)

Startup: group 0 runs as two 512-col half-passes so the first DR matmul
needs only 256 KB resident (chip-level HBM delivery is the startup wall;
the PE must start only once it can stream without gaps, or HAM
re-throttles the clock). DMA rings are urgency-ordered: sync carries the
group-0 pieces and early w8 kts, scalar only what gates the first tanh,
gpsimd the late w8 kts and cold constants.

On-core dataflow per (b, g) group (GW=1024 score columns):
    for kt: psum[k=128, GW] = sum_h w[h,k] enc[h,s]   (clean-mode PE streams)
            th = tanh(psum * (1/WS) + hb[k,b])        (ScalarE, bf16 out)
            acc = th * v[k] + acc                     (DVE fused, f32;
                                                       last kt emits bf16)
    pv[2, 512] per group: one-hot stationary e_sc routes the partition-sum
    of each 512-col half to psum row sc (deferred one group so the PE never
    waits on the DVE chain); ONE exp [2,512] per group with accum_out.
    Last group: accb stops at kt5; kt6/kt7 go tanh-half -> v-one-hot matmul
    straight into pv, so the tail never waits on the DVE chain.
    finish_b: DVE free-reduce of sm2 -> tiny fp32 PE matmul for the
    partition-pair total -> reciprocal -> one [2,1024] scale -> 2 out DMAs.
"""

import numpy as np

B, S, H = 32, 2048, 1024
NCORES = 8
BL = B // NCORES          # batch rows per core
P = 128                   # SBUF partitions
KT = 8                    # k-tiles
K8 = KT
GW = 1024                 # score columns per group (2 PSUM banks)
NG = S // GW              # groups per batch row
WS = 16.0                 # fp8 weight pre-scale

_CACHE = {}


def _build_nc():
    import concourse.bacc as bacc
    import concourse.mybir as mybir
    import concourse.tile as tile

    dt = mybir.dt
    AFT = mybir.ActivationFunctionType
    DR = mybir.MatmulPerfMode.DoubleRow

    nc = bacc.Bacc("TRN2", target_bir_lowering=False, debug=False)

    e8_d = nc.declare_dram_parameter("e8", [BL, NG, P, 4, 2, GW], dt.float8e4, isOutput=False)
    e8a_d = nc.declare_dram_parameter("e8a", [2, 2, P, 2, 2, 512], dt.float8e4, isOutput=False)
    w8_d = nc.declare_dram_parameter("w8", [P, K8, 4, 2, P], dt.float8e4, isOutput=False)
    hb_d = nc.declare_dram_parameter("hb", [P, KT, BL], dt.float32, isOutput=False)
    vvs_d = nc.declare_dram_parameter("vvs", [P, KT], dt.float32, isOutput=False)
    oneh_d = nc.declare_dram_parameter("oneh", [P, 2, 2], dt.bfloat16, isOutput=False)
    vht_d = nc.declare_dram_parameter("vht", [P, 2, 2, 2], dt.bfloat16, isOutput=False)
    on2f_d = nc.declare_dram_parameter("on2f", [2, 2], dt.float32, isOutput=False)
    out_d = nc.declare_dram_parameter("out", [BL, NG, 2, 512], dt.float32, isOutput=True)

    with tile.TileContext(nc) as tc:
        with (
            tc.tile_pool(name="const", bufs=1) as constp,
            tc.tile_pool(name="enc8", bufs=4) as encp8,
            tc.tile_pool(name="enc0", bufs=1) as encp0,
            tc.tile_pool(name="tanh", bufs=4) as tanhp,
            tc.tile_pool(name="accp", bufs=3) as accp,
            tc.tile_pool(name="soft", bufs=2) as softp,
            tc.tile_pool(name="pe", bufs=3, space="PSUM") as pep,
            tc.tile_pool(name="pv", bufs=2, space="PSUM") as pvp,
        ):
            # scalar ring carries only what gates the first tanh (so its
            # queue frees early and the act-table load + tanh chain start
            # on time); everything else rides sync/gpsimd at its deadline
            w8 = constp.tile([P, K8, 4, 2, P], dt.float8e4)
            nc.scalar.dma_start(w8[:, 0], w8_d[:, 0])
            hb = constp.tile([P, KT, BL], dt.float32)
            nc.scalar.dma_start(hb[:], hb_d.ap())
            vvs = constp.tile([P, KT], dt.float32)
            nc.scalar.dma_start(vvs[:], vvs_d.ap())

            # group 0 runs as two 512-col half-passes so the startup demand
            # curve matches chip-level delivery: the first DR matmul needs
            # only 256 KB resident, and every later deadline has >=1 us of
            # margin. Sync-ring order interleaves the early w8 kts at their
            # consumption times.
            e8h = [
                encp0.tile([P, 4, 2, 512], dt.float8e4, tag=f"e8h{i}",
                           name=f"e8h{i}")
                for i in range(2)
            ]
            nc.sync.dma_start(e8h[0][:, 0:2], e8a_d[0][0])
            nc.gpsimd.dma_start(e8h[0][:, 2:4], e8a_d[0][1])
            nc.sync.dma_start(w8[:, 1], w8_d[:, 1])
            nc.sync.dma_start(w8[:, 2], w8_d[:, 2])
            nc.sync.dma_start(e8h[1][:, 0:2], e8a_d[1][0])
            nc.sync.dma_start(e8h[1][:, 2:4], e8a_d[1][1])
            for k8 in range(3, K8):
                nc.gpsimd.dma_start(w8[:, k8], w8_d[:, k8])
            oneh = constp.tile([P, 2, 2], dt.bfloat16)
            nc.gpsimd.dma_start(oneh[:], oneh_d.ap())
            vht = constp.tile([P, 2, 2, 2], dt.bfloat16)
            nc.gpsimd.dma_start(vht[:], vht_d.ap())
            on2f = constp.tile([2, 2], dt.float32)
            nc.gpsimd.dma_start(on2f[:], on2f_d.ap())

            # PE warm-up (clock ramp) while the first DMAs land: cheap
            # [P,1]x[P,512] matmuls into a pe-pool tile.
            wut = constp.tile([P, GW], dt.bfloat16, tag="wut")
            nc.vector.memset(wut[:], 0.0)
            wps = pep.tile([P, GW], dt.float32, tag="ps", name="wps")
            for i in range(6):
                nc.tensor.matmul(
                    wps[0:1, 0:512], wut[:, 0:1],
                    wut[:, 0:512], start=True, stop=True,
                )

            ex_tiles = {}
            sm_tiles = {}
            pending = None

            def finish_group(p):
                pb, pg, pmov, ppv = p
                if ppv is None:
                    pv = pvp.tile([2, 512], dt.float32, tag="pv", name="pv")
                    for sc in range(2):
                        nc.tensor.matmul(
                            pv[:], oneh[:, sc], pmov[sc],
                            start=(sc == 0), stop=(sc == 1),
                        )
                else:
                    pv = ppv
                nc.scalar.activation(
                    ex_tiles[pb][:, pg], pv[:], AFT.Exp,
                    accum_out=sm_tiles[pb][:, pg:pg + 1],
                )

            def finish_b(pb):
                smr = softp.tile([2, 1], dt.float32, tag="smr")
                nc.vector.tensor_reduce(
                    smr[:], sm_tiles[pb][:], axis=mybir.AxisListType.X,
                    op=mybir.AluOpType.add,
                )
                t2 = pvp.tile([2, 1], dt.float32, tag="pv", name="t2")
                nc.tensor.matmul(t2[:], on2f[:], smr[:], start=True, stop=True)
                rc = softp.tile([2, 1], dt.float32, tag="rc")
                nc.vector.reciprocal(rc[:], t2[:])
                ot = softp.tile([2, NG, 512], dt.float32, tag="ot")
                nc.vector.tensor_scalar_mul(ot[:], ex_tiles[pb][:], rc[:])
                nc.gpsimd.dma_start(out_d[pb][0], ot[:, 0])
                nc.sync.dma_start(out_d[pb][1], ot[:, 1])

            for b in range(BL):
                ex_tiles[b] = softp.tile([2, NG, 512], dt.float32, tag="ex", name="ex")
                sm_tiles[b] = softp.tile([2, NG], dt.float32, tag="sm", name="sm")
                for g in range(NG):
                    tsc = 1.0 / WS
                    if b == 0 and g == 0:
                        # two 512-col half-passes (see startup comment)
                        movers = []
                        for hf in range(2):
                            acch = accp.tile([P, GW], dt.float32)
                            accbh = tanhp.tile([P, GW], dt.bfloat16,
                                               tag="accb", name=f"accbh{hf}")
                            for kt in range(KT):
                                ps = pep.tile([P, GW], dt.float32,
                                              tag="ps", name="ps")
                                for blk in range(4):
                                    nc.tensor.matmul(
                                        ps[:, 0:512], w8[:, kt, blk],
                                        e8h[hf][:, blk],
                                        start=(blk == 0), stop=(blk == 3),
                                        perf_mode=DR,
                                    )
                                th = tanhp.tile([P, GW], dt.bfloat16)
                                nc.scalar.activation(
                                    th[:, 0:512], ps[:, 0:512], AFT.Tanh,
                                    bias=hb[:, kt, 0:1], scale=tsc,
                                )
                                if kt == 0:
                                    nc.vector.tensor_scalar_mul(
                                        acch[:, 0:512], th[:, 0:512],
                                        vvs[:, 0:1])
                                else:
                                    dst = accbh if kt == KT - 1 else acch
                                    nc.vector.scalar_tensor_tensor(
                                        dst[:, 0:512], th[:, 0:512],
                                        vvs[:, kt:kt + 1], acch[:, 0:512],
                                        op0=mybir.AluOpType.mult,
                                        op1=mybir.AluOpType.add,
                                    )
                            movers.append(accbh[:, 0:512])
                        pending = (0, 0, movers, None)
                        continue
                    e8t = [
                        encp8.tile([P, 2, GW], dt.float8e4, tag=f"e8t{blk}",
                                   name=f"e8t{blk}")
                        for blk in range(4)
                    ]
                    for blk in range(4):
                        nc.sync.dma_start(e8t[blk][:], e8_d[b][g][:, blk])
                    last = (b == BL - 1 and g == NG - 1)
                    acc = accp.tile([P, GW], dt.float32)
                    accb = tanhp.tile([P, GW], dt.bfloat16, tag="accb", name="accb")
                    if last:
                        pvl = pvp.tile([2, 512], dt.float32, tag="pv", name="pvl")
                        ps6 = None
                    for kt in range(KT):
                        ps = pep.tile([P, GW], dt.float32, tag="ps", name="ps")
                        for blk in range(4):
                            for sc in range(2):
                                nc.tensor.matmul(
                                    ps[:, sc * 512:(sc + 1) * 512],
                                    w8[:, kt, blk],
                                    e8t[blk][:, :, sc * 512:(sc + 1) * 512],
                                    start=(blk == 0), stop=(blk == 3),
                                    perf_mode=DR,
                                )
                        if last and kt == KT - 2:
                            # defer: tanh+v-matmul emitted after kt7's DR
                            # stream so the PE never stalls on the tanh
                            ps6 = ps
                        elif last and kt == KT - 1:
                            # partition-sums of the kt0-5 chain (accb ready
                            # since kt5's DVE op, during kt6's DR stream)
                            for sc in range(2):
                                nc.tensor.matmul(
                                    pvl[:], oneh[:, sc],
                                    accb[:, sc * 512:(sc + 1) * 512],
                                    start=(sc == 0), stop=False,
                                    skip_group_check=True,
                                )
                            # kt6/kt7: one tanh each -> one-hot v-column
                            # matmuls straight into pvl rows
                            for kq, psq in ((KT - 2, ps6), (KT - 1, ps)):
                                thq = tanhp.tile([P, GW], dt.bfloat16,
                                                 tag="thh", name="thh")
                                nc.scalar.activation(
                                    thq[:], psq[:], AFT.Tanh,
                                    bias=hb[:, kq, b:b + 1], scale=tsc,
                                )
                                for sc in range(2):
                                    nc.tensor.matmul(
                                        pvl[:], vht[:, kq - (KT - 2), sc],
                                        thq[:, sc * 512:(sc + 1) * 512],
                                        start=False,
                                        stop=(kq == KT - 1 and sc == 1),
                                        skip_group_check=True,
                                    )
                        else:
                            th = tanhp.tile([P, GW], dt.bfloat16)
                            nc.scalar.activation(
                                th[:], ps[:], AFT.Tanh, bias=hb[:, kt, b:b + 1],
                                scale=tsc,
                            )
                            if kt == 0:
                                nc.vector.tensor_scalar_mul(
                                    acc[:], th[:], vvs[:, kt:kt + 1])
                            else:
                                last_chain = KT - 3 if last else KT - 1
                                dst = accb if kt == last_chain else acc
                                nc.vector.scalar_tensor_tensor(
                                    dst[:], th[:], vvs[:, kt:kt + 1], acc[:],
                                    op0=mybir.AluOpType.mult,
                                    op1=mybir.AluOpType.add,
                                )
                        if kt == 1 and pending is not None:
                            # deferred by a full kt so the pv matmuls never
                            # wait on the previous group's DVE chain
                            finish_group(pending)
                            if pending[1] == NG - 1:
                                finish_b(pending[0])
                            pending = None
                    if last:
                        finish_group((b, g, None, pvl))
                        finish_b(b)
                    else:
                        pending = (b, g, [accb[:, 0:512], accb[:, 512:GW]],
                                   None)

    nc.compile()
    return nc


def _get_nc():
    if "nc" not in _CACHE:
        _CACHE["nc"] = _build_nc()
    return _CACHE["nc"]


def _make_in_maps(hidden, encoder_outputs, attn_W, attn_b, v):
    import concourse.mybir as mybir

    bf16 = mybir.dt.np(mybir.dt.bfloat16)
    f8 = mybir.dt.np(mybir.dt.float8e4)
    f32 = np.float32

    order = np.argsort(-np.abs(v), kind="stable")
    W2p = attn_W[:, H:].T[:, order]          # (h, k) permuted columns
    vp = v[order]
    hid = hidden[0]                           # (B, H)
    hterm = (hid @ attn_W[:, :H].T + attn_b).astype(f32)[:, order]  # (B, k)

    w8 = np.ascontiguousarray(
        (W2p * WS).reshape(4, 2, P, K8, P).transpose(2, 3, 0, 1, 4)
    ).astype(f8)
    vvs = np.ascontiguousarray(vp.reshape(KT, P).T).astype(f32)
    vvb = vvs.astype(bf16)
    oneh = np.zeros((P, 2, 2), dtype=bf16)
    oneh[:, 0, 0] = 1.0
    oneh[:, 1, 1] = 1.0
    vht = np.zeros((P, 2, 2, 2), dtype=bf16)
    for i, kq in enumerate((KT - 2, KT - 1)):
        vht[:, i, 0, 0] = vvb[:, kq]
        vht[:, i, 1, 1] = vvb[:, kq]
    on2f = np.ones((2, 2), dtype=f32)

    shared = {"w8": w8, "vvs": vvs, "oneh": oneh, "vht": vht, "on2f": on2f}

    in_maps = []
    for c in range(NCORES):
        sl = slice(c * BL, (c + 1) * BL)
        encs = encoder_outputs[sl]            # (BL, S, H)
        e8 = np.ascontiguousarray(
            encs.reshape(BL, NG, GW, 4, 2, P).transpose(0, 1, 5, 3, 4, 2)
        ).astype(f8)
        # [sc, blkpair, P, blk%2, dr, col]
        e8a = np.ascontiguousarray(
            encs[0, :GW].reshape(2, 512, 2, 2, 2, P).transpose(0, 2, 5, 3, 4, 1)
        ).astype(f8)
        hbias = np.ascontiguousarray(
            hterm[sl].T.reshape(KT, P, BL).transpose(1, 0, 2)
        )
        m = {"e8": e8, "e8a": e8a, "hb": hbias, **shared}
        in_maps.append(m)
    return in_maps


def kernel(hidden, encoder_outputs, attn_W, attn_b, v):
    from concourse.bass_utils import run_bass_kernel_spmd

    nc = _get_nc()
    in_maps = _make_in_maps(
        np.asarray(hidden, dtype=np.float32),
        np.asarray(encoder_outputs, dtype=np.float32),
        np.asarray(attn_W, dtype=np.float32),
        np.asarray(attn_b, dtype=np.float32),
        np.asarray(v, dtype=np.float32),
    )
    # A freshly-opened device occasionally fails its first execution with
    # NRT_EXEC_UNIT_UNRECOVERABLE; a retry on the reset device succeeds.
    last_err = None
    for attempt in range(3):
        try:
            res = run_bass_kernel_spmd(nc, in_maps, core_ids=list(range(NCORES)))
            break
        except Exception as e:
            last_err = e
            import time
            time.sleep(2.0)
    else:
        raise last_err
    out = np.concatenate(
        [res.results[c]["out"].reshape(BL, S) for c in range(NCORES)], axis=0)
    return out[:, None, :].astype(np.float32)


# revision 4
# speedup vs baseline: 1.0267x; 1.0010x over previous
"""Bahdanau-attention scores kernel for one TRN2 chip (8 NeuronCores).

Reference computation (B=32, S=2048, H=1024):
    energy = tanh(hidden @ W1^T + enc @ W2^T + b)   # (B, S, H)
    scores = energy . v                             # (B, S)
    out    = softmax(scores, axis=S)[:, None, :]    # (B, 1, S)

Distribution: data-parallel over B - each of the 8 cores handles 4 batch
rows; small tensors (attn_W, attn_b, v, hidden) replicated. No collectives.

Numerics: the enc @ W2^T contraction runs entirely in fp8 (e4m3) with
DoubleRow matmuls (2x PE throughput vs bf16). fp8 weights are pre-scaled
by WS=16 (avoids the subnormal region) and the scale is undone by the tanh
activation's input scale; the hidden term is computed exactly on the host.

Per-core layout (pre-packed on host so DMAs are contiguous):
    e8    (BL, NG, P, 4, 2, GW) fp8   enc in DoubleRow layout, h = blk*256+j*128+p
    e8a   (2, 2, P, 2, 2, 512) fp8    group-0 enc, (sc, blkpair) pieces
    w8    (P, K8, 4, 2, P)     fp8    W2^T * WS, DR layout per (kt, blk)
    hb    (P, 8, BL)           f32    hidden @ W1^T + attn_b (host, exact)
    vvs   (P, 8) f32                  v tiled (p, kt)
    oneh  (P, 2, 2) bf16              one-hot columns e0 / e1
    vht   (P, 2, 2, 2) bf16           v-column one-hot [kt-6][sc] -> [P,2]
    on2f  (2, 2) f32                  ones (partition-pair total matmul)

On-core dataflow per (b, g) group (GW=1024 score columns):
    for kt: psum[k=128, GW] = sum_h w[h,k] enc[h,s]   (clean-mode PE streams)
            th = tanh(psum * (1/WS) + hb[k,b])        (ScalarE, bf16 out)
            acc = th * v[k] + acc                     (DVE fused, f32;
                                                       last kt emits bf16)
    pv[2, 512] per group: one-hot stationary e_sc routes the partition-sum
    of each 512-col half to psum row sc (deferred one group so the PE never
    waits on the DVE chain); ONE exp [2,512] per group with accum_out.
    Last group: accb stops at kt5; kt6/kt7 go tanh-half -> v-one-hot matmul
    straight into pv, so the tail never waits on the DVE chain.
    finish_b: DVE free-reduce of sm2 -> tiny fp32 PE matmul for the
    partition-pair total -> reciprocal -> one [2,1024] scale (Scalar Copy
    for streamed rows so the DVE chain never sees a burst; DVE on the
    latency-critical last row) -> 2 out DMAs.

Startup: group 0 runs as two 512-col half-passes so the first DR matmul
needs only 256 KB resident (chip-level HBM delivery is the startup wall;
starting the PE into starvation triggers HAM re-throttle). DMA rings are
urgency-ordered: sync carries group-0 pieces + early w8 kts, scalar only
what gates the first tanh, gpsimd the late w8 kts and cold constants.
"""

import numpy as np

B, S, H = 32, 2048, 1024
NCORES = 8
BL = B // NCORES          # batch rows per core
P = 128                   # SBUF partitions
KT = 8                    # k-tiles
K8 = KT
GW = 1024                 # score columns per group (2 PSUM banks)
NG = S // GW              # groups per batch row
WS = 16.0                 # fp8 weight pre-scale

_CACHE = {}


def _build_nc():
    import concourse.bacc as bacc
    import concourse.mybir as mybir
    import concourse.tile as tile

    dt = mybir.dt
    AFT = mybir.ActivationFunctionType
    DR = mybir.MatmulPerfMode.DoubleRow

    nc = bacc.Bacc("TRN2", target_bir_lowering=False, debug=False)

    e8_d = nc.declare_dram_parameter("e8", [BL, NG, P, 4, 2, GW], dt.float8e4, isOutput=False)
    e8a_d = nc.declare_dram_parameter("e8a", [2, 2, P, 2, 2, 512], dt.float8e4, isOutput=False)
    w8_d = nc.declare_dram_parameter("w8", [P, K8, 4, 2, P], dt.float8e4, isOutput=False)
    hb_d = nc.declare_dram_parameter("hb", [P, KT, BL], dt.float32, isOutput=False)
    vvs_d = nc.declare_dram_parameter("vvs", [P, KT], dt.float32, isOutput=False)
    oneh_d = nc.declare_dram_parameter("oneh", [P, 2, 2], dt.bfloat16, isOutput=False)
    vht_d = nc.declare_dram_parameter("vht", [P, 2, 2, 2], dt.bfloat16, isOutput=False)
    on2f_d = nc.declare_dram_parameter("on2f", [2, 2], dt.float32, isOutput=False)
    out_d = nc.declare_dram_parameter("out", [BL, NG, 2, 512], dt.float32, isOutput=True)

    with tile.TileContext(nc) as tc:
        with (
            tc.tile_pool(name="const", bufs=1) as constp,
            tc.tile_pool(name="enc8", bufs=4) as encp8,
            tc.tile_pool(name="enc0", bufs=1) as encp0,
            tc.tile_pool(name="tanh", bufs=4) as tanhp,
            tc.tile_pool(name="accp", bufs=3) as accp,
            tc.tile_pool(name="soft", bufs=2) as softp,
            tc.tile_pool(name="pe", bufs=3, space="PSUM") as pep,
            tc.tile_pool(name="pv", bufs=2, space="PSUM") as pvp,
        ):
            # scalar ring carries only what gates the first tanh (so its
            # queue frees early and the act-table load + tanh chain start
            # on time); everything else rides sync/gpsimd at its deadline
            w8 = constp.tile([P, K8, 4, 2, P], dt.float8e4)
            nc.scalar.dma_start(w8[:, 0], w8_d[:, 0])
            hb = constp.tile([P, KT, BL], dt.float32)
            nc.scalar.dma_start(hb[:], hb_d.ap())
            vvs = constp.tile([P, KT], dt.float32)
            nc.scalar.dma_start(vvs[:], vvs_d.ap())

            # group 0 runs as two 512-col half-passes so the startup demand
            # curve matches chip-level delivery: the first DR matmul needs
            # only 256 KB resident, and every later deadline has >=1 us of
            # margin. Sync-ring order interleaves the early w8 kts at their
            # consumption times.
            e8h = [
                encp0.tile([P, 4, 2, 512], dt.float8e4, tag=f"e8h{i}",
                           name=f"e8h{i}")
                for i in range(2)
            ]
            nc.sync.dma_start(e8h[0][:, 0:2], e8a_d[0][0])
            nc.gpsimd.dma_start(e8h[0][:, 2:4], e8a_d[0][1])
            nc.sync.dma_start(w8[:, 1], w8_d[:, 1])
            nc.sync.dma_start(w8[:, 2], w8_d[:, 2])
            nc.sync.dma_start(e8h[1][:, 0:2], e8a_d[1][0])
            nc.sync.dma_start(e8h[1][:, 2:4], e8a_d[1][1])
            for k8 in range(3, K8):
                nc.gpsimd.dma_start(w8[:, k8], w8_d[:, k8])
            oneh = constp.tile([P, 2, 2], dt.bfloat16)
            nc.gpsimd.dma_start(oneh[:], oneh_d.ap())
            vht = constp.tile([P, 2, 2, 2], dt.bfloat16)
            nc.gpsimd.dma_start(vht[:], vht_d.ap())
            on2f = constp.tile([2, 2], dt.float32)
            nc.gpsimd.dma_start(on2f[:], on2f_d.ap())

            # PE warm-up (clock ramp) while the first DMAs land: cheap
            # [P,1]x[P,512] matmuls into a pe-pool tile.
            wut = constp.tile([P, 512], dt.bfloat16, tag="wut")
            nc.vector.memset(wut[:], 0.0)
            wps = pep.tile([P, GW], dt.float32, tag="ps", name="wps")
            for i in range(7):
                nc.tensor.matmul(
                    wps[0:1, 0:512], wut[:, 0:1],
                    wut[:], start=True, stop=True,
                )

            ex_tiles = {}
            sm_tiles = {}
            pending = None

            def finish_group(p):
                pb, pg, pmov, ppv = p
                if ppv is None:
                    pv = pvp.tile([2, 512], dt.float32, tag="pv", name="pv")
                    for sc in range(2):
                        nc.tensor.matmul(
                            pv[:], oneh[:, sc], pmov[sc],
                            start=(sc == 0), stop=(sc == 1),
                        )
                else:
                    pv = ppv
                nc.scalar.activation(
                    ex_tiles[pb][:, pg], pv[:], AFT.Exp,
                    accum_out=sm_tiles[pb][:, pg:pg + 1],
                )

            def finish_b(pb, last_b=False):
                smr = softp.tile([2, 1], dt.float32, tag="smr")
                nc.vector.tensor_reduce(
                    smr[:], sm_tiles[pb][:], axis=mybir.AxisListType.X,
                    op=mybir.AluOpType.add,
                )
                t2 = pvp.tile([2, 1], dt.float32, tag="pv", name="t2")
                nc.tensor.matmul(t2[:], on2f[:], smr[:], start=True, stop=True)
                rc = softp.tile([2, 1], dt.float32, tag="rc")
                nc.vector.reciprocal(rc[:], t2[:])
                ot = softp.tile([2, NG, 512], dt.float32, tag="ot")
                if last_b:
                    # latency path: DVE mul is faster than scalar Copy and
                    # both out-DMAs ride the fast sync ring back-to-back
                    nc.vector.tensor_scalar_mul(ot[:], ex_tiles[pb][:], rc[:])
                    nc.sync.dma_start(out_d[pb][0], ot[:, 0])
                    nc.sync.dma_start(out_d[pb][1], ot[:, 1])
                else:
                    # throughput path: the scale rides the Scalar engine
                    # (4us/group slack) so the DVE stt chain is never
                    # delayed by a softmax-finish burst
                    nc.scalar.activation(ot[:], ex_tiles[pb][:], AFT.Copy,
                                         scale=rc[:])
                    nc.gpsimd.dma_start(out_d[pb][0], ot[:, 0])
                    nc.sync.dma_start(out_d[pb][1], ot[:, 1])

            for b in range(BL):
                ex_tiles[b] = softp.tile([2, NG, 512], dt.float32, tag="ex", name="ex")
                sm_tiles[b] = softp.tile([2, NG], dt.float32, tag="sm", name="sm")
                for g in range(NG):
                    tsc = 1.0 / WS
                    if b == 0 and g == 0:
                        # two 512-col half-passes (see startup comment)
                        movers = []
                        for hf in range(2):
                            acch = accp.tile([P, GW], dt.float32)
                            accbh = tanhp.tile([P, GW], dt.bfloat16,
                                               tag="accb", name=f"accbh{hf}")
                            for kt in range(KT):
                                ps = pep.tile([P, GW], dt.float32,
                                              tag="ps", name="ps")
                                for blk in range(4):
                                    nc.tensor.matmul(
                                        ps[:, 0:512], w8[:, kt, blk],
                                        e8h[hf][:, blk],
                                        start=(blk == 0), stop=(blk == 3),
                                        perf_mode=DR,
                                    )
                                th = tanhp.tile([P, GW], dt.bfloat16)
                                nc.scalar.activation(
                                    th[:, 0:512], ps[:, 0:512], AFT.Tanh,
                                    bias=hb[:, kt, 0:1], scale=tsc,
                                )
                                if kt == 0:
                                    nc.vector.tensor_scalar_mul(
                                        acch[:, 0:512], th[:, 0:512],
                                        vvs[:, 0:1])
                                else:
                                    dst = accbh if kt == KT - 1 else acch
                                    nc.vector.scalar_tensor_tensor(
                                        dst[:, 0:512], th[:, 0:512],
                                        vvs[:, kt:kt + 1], acch[:, 0:512],
                                        op0=mybir.AluOpType.mult,
                                        op1=mybir.AluOpType.add,
                                    )
                            movers.append(accbh[:, 0:512])
                        pending = (0, 0, movers, None)
                        continue
                    e8t = [
                        encp8.tile([P, 2, GW], dt.float8e4, tag=f"e8t{blk}",
                                   name=f"e8t{blk}")
                        for blk in range(4)
                    ]
                    for blk in range(4):
                        nc.sync.dma_start(e8t[blk][:], e8_d[b][g][:, blk])
                    last = (b == BL - 1 and g == NG - 1)
                    acc = accp.tile([P, GW], dt.float32)
                    accb = tanhp.tile([P, GW], dt.bfloat16, tag="accb", name="accb")
                    if last:
                        pvl = pvp.tile([2, 512], dt.float32, tag="pv", name="pvl")
                        ps6 = None
                    for kt in range(KT):
                        ps = pep.tile([P, GW], dt.float32, tag="ps", name="ps")
                        for blk in range(4):
                            for sc in range(2):
                                nc.tensor.matmul(
                                    ps[:, sc * 512:(sc + 1) * 512],
                                    w8[:, kt, blk],
                                    e8t[blk][:, :, sc * 512:(sc + 1) * 512],
                                    start=(blk == 0), stop=(blk == 3),
                                    perf_mode=DR,
                                )
                        if last and kt == KT - 2:
                            # defer: tanh+v-matmul emitted after kt7's DR
                            # stream so the PE never stalls on the tanh
                            ps6 = ps
                        elif last and kt == KT - 1:
                            # partition-sums of the kt0-5 chain (accb ready
                            # since kt5's DVE op, during kt6's DR stream)
                            for sc in range(2):
                                nc.tensor.matmul(
                                    pvl[:], oneh[:, sc],
                                    accb[:, sc * 512:(sc + 1) * 512],
                                    start=(sc == 0), stop=False,
                                    skip_group_check=True,
                                )
                            # kt6/kt7: one tanh each -> one-hot v-column
                            # matmuls straight into pvl rows
                            for kq, psq in ((KT - 2, ps6), (KT - 1, ps)):
                                thq = tanhp.tile([P, GW], dt.bfloat16,
                                                 tag="thh", name="thh")
                                nc.scalar.activation(
                                    thq[:], psq[:], AFT.Tanh,
                                    bias=hb[:, kq, b:b + 1], scale=tsc,
                                )
                                for sc in range(2):
                                    nc.tensor.matmul(
                                        pvl[:], vht[:, kq - (KT - 2), sc],
                                        thq[:, sc * 512:(sc + 1) * 512],
                                        start=False,
                                        stop=(kq == KT - 1 and sc == 1),
                                        skip_group_check=True,
                                    )
                        else:
                            th = tanhp.tile([P, GW], dt.bfloat16)
                            nc.scalar.activation(
                                th[:], ps[:], AFT.Tanh, bias=hb[:, kt, b:b + 1],
                                scale=tsc,
                            )
                            if kt == 0:
                                nc.vector.tensor_scalar_mul(
                                    acc[:], th[:], vvs[:, kt:kt + 1])
                            else:
                                last_chain = KT - 3 if last else KT - 1
                                dst = accb if kt == last_chain else acc
                                nc.vector.scalar_tensor_tensor(
                                    dst[:], th[:], vvs[:, kt:kt + 1], acc[:],
                                    op0=mybir.AluOpType.mult,
                                    op1=mybir.AluOpType.add,
                                )
                        if kt == 2 and pending is not None:
                            # deferred by a full kt so the pv matmuls never
                            # wait on the previous group's DVE chain
                            finish_group(pending)
                            if pending[1] == NG - 1:
                                finish_b(pending[0])
                            pending = None
                    if last:
                        finish_group((b, g, None, pvl))
                        finish_b(b, last_b=True)
                    else:
                        pending = (b, g, [accb[:, 0:512], accb[:, 512:GW]],
                                   None)

    nc.compile()
    return nc


def _get_nc():
    if "nc" not in _CACHE:
        _CACHE["nc"] = _build_nc()
    return _CACHE["nc"]


def _make_in_maps(hidden, encoder_outputs, attn_W, attn_b, v):
    import concourse.mybir as mybir

    bf16 = mybir.dt.np(mybir.dt.bfloat16)
    f8 = mybir.dt.np(mybir.dt.float8e4)
    f32 = np.float32

    order = np.argsort(-np.abs(v), kind="stable")
    W2p = attn_W[:, H:].T[:, order]          # (h, k) permuted columns
    vp = v[order]
    hid = hidden[0]                           # (B, H)
    hterm = (hid @ attn_W[:, :H].T + attn_b).astype(f32)[:, order]  # (B, k)

    w8 = np.ascontiguousarray(
        (W2p * WS).reshape(4, 2, P, K8, P).transpose(2, 3, 0, 1, 4)
    ).astype(f8)
    vvs = np.ascontiguousarray(vp.reshape(KT, P).T).astype(f32)
    vvb = vvs.astype(bf16)
    oneh = np.zeros((P, 2, 2), dtype=bf16)
    oneh[:, 0, 0] = 1.0
    oneh[:, 1, 1] = 1.0
    vht = np.zeros((P, 2, 2, 2), dtype=bf16)
    for i, kq in enumerate((KT - 2, KT - 1)):
        vht[:, i, 0, 0] = vvb[:, kq]
        vht[:, i, 1, 1] = vvb[:, kq]
    on2f = np.ones((2, 2), dtype=f32)

    shared = {"w8": w8, "vvs": vvs, "oneh": oneh, "vht": vht, "on2f": on2f}

    in_maps = []
    for c in range(NCORES):
        sl = slice(c * BL, (c + 1) * BL)
        encs = encoder_outputs[sl]            # (BL, S, H)
        e8 = np.ascontiguousarray(
            encs.reshape(BL, NG, GW, 4, 2, P).transpose(0, 1, 5, 3, 4, 2)
        ).astype(f8)
        # [sc, blkpair, P, blk%2, dr, col]
        e8a = np.ascontiguousarray(
            encs[0, :GW].reshape(2, 512, 2, 2, 2, P).transpose(0, 2, 5, 3, 4, 1)
        ).astype(f8)
        hbias = np.ascontiguousarray(
            hterm[sl].T.reshape(KT, P, BL).transpose(1, 0, 2)
        )
        m = {"e8": e8, "e8a": e8a, "hb": hbias, **shared}
        in_maps.append(m)
    return in_maps


def kernel(hidden, encoder_outputs, attn_W, attn_b, v):
    from concourse.bass_utils import run_bass_kernel_spmd

    nc = _get_nc()
    in_maps = _make_in_maps(
        np.asarray(hidden, dtype=np.float32),
        np.asarray(encoder_outputs, dtype=np.float32),
        np.asarray(attn_W, dtype=np.float32),
        np.asarray(attn_b, dtype=np.float32),
        np.asarray(v, dtype=np.float32),
    )
    # A freshly-opened device occasionally fails its first execution with
    # NRT_EXEC_UNIT_UNRECOVERABLE; a retry on the reset device succeeds.
    last_err = None
    for attempt in range(3):
        try:
            res = run_bass_kernel_spmd(nc, in_maps, core_ids=list(range(NCORES)))
            break
        except Exception as e:
            last_err = e
            import time
            time.sleep(2.0)
    else:
        raise last_err
    out = np.concatenate(
        [res.results[c]["out"].reshape(BL, S) for c in range(NCORES)], axis=0)
    return out[:, None, :].astype(np.float32)
